# revision 4
# baseline (speedup 1.0000x reference)
"""Fused masked-attention v2.2 for Trainium2, data-parallel over batch on 8 cores.

Per core (one batch, L=4096, E=1024, H=64):
  psum S = (a/2)*raw_scores  (a/2 = 0.5*0.125*1024*log2e folded into WQ; scores
           row-tiled K=64 pairs via tile_position, even/odd chunk layout in KT)
  y1 = int16(trunc(S + 15104) * notm)          -- DVE scalar_tensor_tensor
  y2 = y1 + 512 (integer add via fp32 ALU)     -- ACT / GPSIMD alternating
  P  = y1.as_fp16 * y2.as_fp16                 -- DVE (2x packed f16 mode)
       == exp(0.125*raw)*const*(1 + eps), eps ~ 0.5% (half-period pair product
       cancels the Schraudolph sawtooth's first harmonic; masked -> +0.0)
  O  = (P^T V_ext)/Z, V_ext=[V|1] fp16; AV uses P slices as FWL stationaries so
       output lands directly as [q, h+Z]; Z at col 64; epilogue = 1/Z * row.

HBM per core: q/k fp8e4m3 (host cast), v fp16, mask u8 {0,1} = 33MB, all HWDGE.
"""

import numpy as np
import ml_dtypes

import concourse.bass as bass
import concourse.tile as tile
from concourse import bacc, mybir
from concourse import bass_utils

B, L, E, H = 8, 4096, 1024, 64
NCORES = 8
F32 = mybir.dt.float32
F16 = mybir.dt.float16
F8 = mybir.dt.float8e4
I16 = mybir.dt.int16
U8 = mybir.dt.uint8

LB = 512
NQB = L // LB
B1 = 15104.0
IDENT = mybir.ActivationFunctionType.Identity
DEBUG_DUMP = False


def build_nc():
    nc = bacc.Bacc(
        "TRN2",
        target_bir_lowering=False,
        debug=False,
        enable_asserts=False,
        num_devices=NCORES,
    )
    qT8 = nc.dram_tensor("qT8", [E, L], F8, kind="ExternalInput").ap()
    kT8 = nc.dram_tensor("kT8", [E, L], F8, kind="ExternalInput").ap()
    vT16 = nc.dram_tensor("vT16", [E, L], F16, kind="ExternalInput").ap()
    mpl8 = nc.dram_tensor("mpl8", [L, L], F8, kind="ExternalInput").ap()
    ident8 = nc.dram_tensor("ident8", [128, 128], F8, kind="ExternalInput").ap()
    wqT2 = nc.dram_tensor("wqT2", [E, 128], F16, kind="ExternalInput").ap()
    wkT2 = nc.dram_tensor("wkT2", [E, 128], F16, kind="ExternalInput").ap()
    wvT = nc.dram_tensor("wvT", [E, H], F16, kind="ExternalInput").ap()
    out = nc.dram_tensor("out", [L, H], F32, kind="ExternalOutput").ap()
    if DEBUG_DUMP:
        d_qt = nc.dram_tensor("d_qt", [128, L], F16, kind="ExternalOutput").ap()
        d_kt = nc.dram_tensor("d_kt", [128, L // 2], F16, kind="ExternalOutput").ap()
        d_v = nc.dram_tensor("d_v", [128, (L // 128) * 66], F16, kind="ExternalOutput").ap()
        d_pst = nc.dram_tensor("d_pst", [128, 1024], F32, kind="ExternalOutput").ap()
        d_y1 = nc.dram_tensor("d_y1", [128, 1024], I16, kind="ExternalOutput").ap()
        d_pp = nc.dram_tensor("d_pp", [128, 1024], F16, kind="ExternalOutput").ap()
        d_poq = nc.dram_tensor("d_poq", [128, 264], F32, kind="ExternalOutput").ap()

    qT_r = qT8.rearrange("(c p) l -> p c l", p=128)
    kT_r = kT8.rearrange("(c p) l -> p c l", p=128)
    vT_r = vT16.rearrange("(c p) l -> p c l", p=128)
    mpl_r = mpl8.rearrange("(c p) q -> p c q", p=128)
    ADD = mybir.AluOpType.add
    MULT = mybir.AluOpType.mult

    with tile.TileContext(nc) as tc:
        with (
            tc.tile_pool(name="const", bufs=1) as constp,
            tc.tile_pool(name="persist", bufs=1) as persist,
            tc.tile_pool(name="kin", bufs=3) as kinp,
            tc.tile_pool(name="vin", bufs=2) as vinp,
            tc.tile_pool(name="qin", bufs=3) as qinp,
            tc.tile_pool(name="mask", bufs=2) as mpool,
            tc.tile_pool(name="y1p", bufs=3) as y1pool,
            tc.tile_pool(name="y2p", bufs=3) as y2pool,
            tc.tile_pool(name="pp", bufs=4) as ppool,
            tc.tile_pool(name="zinv", bufs=4) as zpool,
            tc.tile_pool(name="otile", bufs=4) as otpool,
            tc.tile_pool(name="ps_st", bufs=2, space="PSUM") as ps_st,
            tc.tile_pool(name="ps_oq", bufs=2, space="PSUM") as ps_oq,
            tc.tile_pool(name="ps_small", bufs=2, space="PSUM") as ps_small,
        ):
            wq_sb = constp.tile([128, 8, 128], F16)
            wk_sb = constp.tile([128, 8, 128], F16)
            wv_sb = constp.tile([128, 8, H], F16)
            id8_sb = constp.tile([128, 128], F8)
            nc.sync.dma_start(id8_sb[:], ident8)
            nc.sync.dma_start(wq_sb[:], wqT2.rearrange("(c p) h -> p c h", p=128))
            nc.sync.dma_start(wk_sb[:], wkT2.rearrange("(c p) h -> p c h", p=128))
            nc.sync.dma_start(wv_sb[:], wvT.rearrange("(c p) h -> p c h", p=128))

            QT_sb = persist.tile([128, L], F16)
            KT_sb = persist.tile([128, L // 2], F16)
            V_sb = persist.tile([128, L // 128, 66], F16)
            nc.vector.memset(V_sb[:, :, 64:66], 0.0)
            nc.vector.memset(V_sb[:, :, 64:65], 1.0)

            # ---------------- Phase 1: K then V ----------------
            for lb in range(NQB):
                ls = lb * LB
                k_in = kinp.tile([128, 8, LB], F8, tag="kin")
                nc.sync.dma_start(k_in[:], kT_r[:, :, ls : ls + LB])
                p_k = ps_small.tile([128, LB], F32, tag="small")
                for ec in range(8):
                    nc.tensor.matmul(
                        p_k[:], wk_sb[:, ec, :], k_in[:, ec, :],
                        start=(ec == 0), stop=(ec == 7),
                    )
                pk_e = p_k[0:64, :].rearrange("p (c x) -> p c x", x=128)
                pk_o = p_k[64:128, :].rearrange("p (c x) -> p c x", x=128)
                kt_e = KT_sb[0:64, 256 * lb : 256 * lb + 256].rearrange(
                    "p (c x) -> p c x", x=128
                )
                kt_o = KT_sb[64:128, 256 * lb : 256 * lb + 256].rearrange(
                    "p (c x) -> p c x", x=128
                )
                nc.vector.tensor_scalar_mul(kt_e, pk_e[:, 0::2, :], 1.0)
                nc.vector.tensor_scalar_mul(kt_o, pk_o[:, 1::2, :], 1.0)
            for lb in range(NQB):
                ls = lb * LB
                v_in = vinp.tile([128, 8, LB], F16, tag="vin")
                nc.sync.dma_start(v_in[:], vT_r[:, :, ls : ls + LB])
                for sub in range(4):
                    p_v = ps_small.tile([128, H], F32, tag="small")
                    for ec in range(8):
                        nc.tensor.matmul(
                            p_v[:],
                            v_in[:, ec, sub * 128 : (sub + 1) * 128],
                            wv_sb[:, ec, :],
                            start=(ec == 0),
                            stop=(ec == 7),
                        )
                    nc.vector.tensor_scalar_mul(V_sb[:, 4 * lb + sub, 0:H], p_v[:], 1.0)

            # ---------------- Phase 2 helpers ----------------
            def load_q(qb):
                qs = qb * LB
                q_in = qinp.tile([128, 8, LB], F8, tag="qin")
                nc.sync.dma_start(q_in[:], qT_r[:, :, qs : qs + LB])
                return q_in

            def proj_q(qb, q_in):
                qs = qb * LB
                p_q = ps_small.tile([128, LB], F32, tag="small")
                for ec in range(8):
                    nc.tensor.matmul(
                        p_q[:], wq_sb[:, ec, :], q_in[:, ec, :],
                        start=(ec == 0), stop=(ec == 7),
                    )
                nc.vector.tensor_scalar_mul(QT_sb[:, qs : qs + LB], p_q[:], 1.0)

            def alloc_mask():
                return mpool.tile([128, L // 128, LB], F8, tag="m", name="mtile")

            def load_mask_quarter(m, qb, qtr):
                qs = qb * LB
                nc.sync.dma_start(
                    m[:, 8 * qtr : 8 * (qtr + 1), :],
                    mpl_r[:, 8 * qtr : 8 * (qtr + 1), qs : qs + LB],
                )

            def epilogue(qb, p_oq):
                for qsub in range(4):
                    zinv = zpool.tile([128, 1], F32, tag="z")
                    nc.vector.reciprocal(zinv[:], p_oq[:, qsub, 64:65])
                    ot = otpool.tile([128, H], F32, tag="ot")
                    nc.vector.tensor_scalar_mul(ot[:], p_oq[:, qsub, 0:H], zinv[:])
                    r0 = qb * LB + qsub * 128
                    nc.sync.dma_start(out[r0 : r0 + 128, :], ot[:])

            q0 = load_q(0)
            m0 = alloc_mask()
            for qtr in range(4):
                load_mask_quarter(m0, 0, qtr)
            proj_q(0, q0)
            q_next = load_q(1)

            # HAM warmup burst
            p_w = ps_st.tile([128, 1024], F32, tag="p_st")
            for w in range(32):
                nc.tensor.matmul(
                    p_w[:, 0:128], KT_sb[0:64, 0:128], QT_sb[0:64, 0:128],
                    start=True, stop=True, tile_position=(0, 0),
                )

            pending = None
            mtile = m0
            for qb in range(NQB):
                qs = qb * LB
                p_oq = ps_oq.tile([128, 4, 66], F32, tag="p_oq")
                m_next = None
                for j in range(16):
                    p_st = ps_st.tile([128, 1024], F32, tag="p_st")
                    nc.tensor.matmul(
                        p_st[:, 0:LB],
                        KT_sb[0:64, j * 128 : (j + 1) * 128],
                        QT_sb[0:64, qs : qs + LB],
                        start=True, stop=False, tile_position=(0, 0),
                        skip_group_check=True,
                    )
                    nc.tensor.matmul(
                        p_st[:, LB : 2 * LB],
                        KT_sb[64:128, j * 128 : (j + 1) * 128],
                        QT_sb[64:128, qs : qs + LB],
                        start=True, stop=False, tile_position=(64, 0),
                        skip_group_check=True,
                    )
                    if j == 1 and pending is not None:
                        epilogue(*pending)
                        pending = None
                    if qb + 1 < NQB and j in (3, 5, 7, 9):
                        if m_next is None:
                            m_next = alloc_mask()
                        load_mask_quarter(m_next, qb + 1, (j - 3) // 2)
                    if j == 8 and qb + 1 < NQB:
                        proj_q(qb + 1, q_next)
                    if j == 12 and qb + 2 < NQB:
                        q_next = load_q(qb + 2)

                    for half in range(2):
                        for t in range(4):
                            nc.tensor.matmul(
                                p_st[32 * t : 32 * t + 32, half * LB : (half + 1) * LB],
                                id8_sb[32 * t : 32 * t + 32, 32 * t : 32 * t + 32],
                                mtile[32 * t : 32 * t + 32, 2 * j + half, :],
                                start=False, stop=True,
                                tile_position=(32 * t, 32 * t),
                                skip_group_check=True,
                            )
                    pp = ppool.tile([128, 2 * LB], F16, tag="pp")
                    nc.scalar.activation(
                        pp[:], p_st[:], mybir.ActivationFunctionType.Exp, scale=0.125
                    )
                    if DEBUG_DUMP and qb == 0 and j == 0:
                        dbg_pst = persist.tile([128, 1024], F32, name="dbg_pst")
                        nc.scalar.copy(dbg_pst[:], p_st[:])
                        nc.sync.dma_start(d_pst, dbg_pst[:])
                        nc.sync.dma_start(d_y1, y1[:])
                        nc.sync.dma_start(d_pp, pp[:])
                    for qsub in range(4):
                        # start=True clears the WHOLE psum bank (first_mm), so
                        # only the very first MM of this shared-bank tile may
                        # set it; later regions overwrite-on-first-touch via
                        # their cleared has_written bits.
                        nc.tensor.matmul(
                            p_oq[:, qsub, :],
                            pp[:, qsub * 128 : (qsub + 1) * 128],
                            V_sb[:, 2 * j, :],
                            start=(j == 0 and qsub == 0), stop=False,
                            skip_group_check=True,
                        )
                        nc.tensor.matmul(
                            p_oq[:, qsub, :],
                            pp[:, LB + qsub * 128 : LB + (qsub + 1) * 128],
                            V_sb[:, 2 * j + 1, :],
                            start=False, stop=(j == 15),
                            skip_group_check=True,
                        )
                if DEBUG_DUMP and qb == 0:
                    dbg_poq = persist.tile([128, 264], F32, name="dbg_poq")
                    nc.scalar.copy(dbg_poq[:], p_oq[:].rearrange("p a b -> p (a b)"))
                    nc.sync.dma_start(d_poq, dbg_poq[:])
                pending = (qb, p_oq)
                mtile = m_next
            epilogue(*pending)
            if DEBUG_DUMP:
                nc.sync.dma_start(d_qt, QT_sb[:])
                nc.sync.dma_start(d_kt, KT_sb[:])
                nc.sync.dma_start(d_v, V_sb[:].rearrange("p a b -> p (a b)"))
    nc.compile()
    return nc


_NC_CACHE = {}


def kernel(query, key, value, mask, WQ, WK, WV):
    if "nc" not in _NC_CACHE:
        _NC_CACHE["nc"] = build_nc()
    nc = _NC_CACHE["nc"]

    f8 = ml_dtypes.float8_e4m3
    wq2 = np.asarray(WQ, np.float32).T
    wq2 = np.ascontiguousarray(np.concatenate([wq2, wq2], axis=1).astype(np.float16))
    wk2 = np.asarray(WK, np.float32).T
    wk2 = np.ascontiguousarray(np.concatenate([wk2, wk2], axis=1).astype(np.float16))
    wvt = np.ascontiguousarray(np.asarray(WV, np.float16).T)
    # fp8 mask plane: masked -> -128.0 (byte 0xF0, same in e4m3fn/ieee), else 0
    mplane = np.where(np.asarray(mask), np.uint8(0xF0), np.uint8(0)).astype(np.uint8)
    id8 = np.eye(128, dtype=np.float32) * 2.0
    id8 = id8.astype(f8)
    in_maps = []
    for b in range(B):
        in_maps.append(
            {
                "qT8": np.ascontiguousarray(np.asarray(query[b]).T).astype(f8),
                "kT8": np.ascontiguousarray(np.asarray(key[b]).T).astype(f8),
                "vT16": np.ascontiguousarray(np.asarray(value[b], np.float16).T),
                "mpl8": np.ascontiguousarray(mplane[b].T).view(f8),
                "ident8": id8,
                "wqT2": wq2,
                "wkT2": wk2,
                "wvT": wvt,
            }
        )
    res = bass_utils.run_bass_kernel_spmd(nc, in_maps, core_ids=list(range(NCORES)))
    return np.stack([res.results[b]["out"] for b in range(B)], axis=0)


if __name__ == "__main__":
    rng = np.random.default_rng(0)
    q = rng.standard_normal((B, L, E), dtype=np.float32)
    k = rng.standard_normal((B, L, E), dtype=np.float32)
    v = rng.standard_normal((B, L, E), dtype=np.float32)
    m = rng.integers(0, 2, size=(B, L, L)).astype(bool)
    s = 1.0 / np.sqrt(E)
    wq = rng.uniform(-s, s, size=(H, E)).astype(np.float32)
    wk = rng.uniform(-s, s, size=(H, E)).astype(np.float32)
    wv = rng.uniform(-s, s, size=(H, E)).astype(np.float32)
    o = kernel(query=q, key=k, value=v, mask=m, WQ=wq, WK=wk, WV=wv)
    print(o.shape, o.dtype)


# revision 23
# speedup vs baseline: 1.8149x; 1.8149x over previous
"""Fused masked-attention for Trainium2, data-parallel over batch on 8 cores.

Per core (one batch element, L=4096, E=1024, H=64):
  Q/K projections from fp8e4m3 inputs (host cast, free); V from fp16 input.
  Weights fp16, host-duplicated [W^T|W^T] so the score matmuls' row-tiled
  halves (tile_position (0,0)/(64,0), K=64 each, even/odd chunk layout in
  KT_sb) read their stationary/moving operands at the right partitions.
  Scores accumulate in PSUM f32; ACT computes exp(0.125*s) -> fp16; the
  mask arrives as u8 {0,1}, SWDGE-cast to fp16 during DMA, and is applied
  by one DVE f16x f16 multiply (2x packed mode). AV uses V_ext=[V|1] fp16
  as stationary (out [h+Z, q]); AV matmuls are software-pipelined 2 pairs
  behind exp so the in-order PE queue never stalls on ACT. Epilogue:
  PE transpose to [q, h+Z], DVE reciprocal of Z, scale, DMA out.

HBM per core: q/k fp8 (4MB each), v fp16 (8MB), mask u8 (16MB), out 1MB.
"""

import numpy as np
import ml_dtypes

import concourse.bass as bass
import concourse.tile as tile
from concourse import bacc, mybir
from concourse import bass_utils

B, L, E, H = 8, 4096, 1024, 64
NCORES = 8
F32 = mybir.dt.float32
F16 = mybir.dt.float16
F8 = mybir.dt.float8e4
I16 = mybir.dt.int16
U8 = mybir.dt.uint8

LB = 512
NQB = L // LB
B1 = 15104.0
IDENT = mybir.ActivationFunctionType.Identity
DEBUG_DUMP = False


def build_nc():
    nc = bacc.Bacc(
        "TRN2",
        target_bir_lowering=False,
        debug=False,
        enable_asserts=False,
        num_devices=NCORES,
    )
    qP = nc.dram_tensor("qP", [128, 8, 8, LB], F8, kind="ExternalInput").ap()
    kP = nc.dram_tensor("kP", [128, 8, 8, LB], F8, kind="ExternalInput").ap()
    vP = nc.dram_tensor("vP", [128, 8, 8, LB], F16, kind="ExternalInput").ap()
    mP = nc.dram_tensor("mP", [128, 8, 32, LB], U8, kind="ExternalInput").ap()
    wqP = nc.dram_tensor("wqP", [128, 8 * 128], F16, kind="ExternalInput").ap()
    wkP = nc.dram_tensor("wkP", [128, 8 * 128], F16, kind="ExternalInput").ap()
    wvP = nc.dram_tensor("wvP", [128, 8 * H], F16, kind="ExternalInput").ap()
    out = nc.dram_tensor("out", [L, H], F32, kind="ExternalOutput").ap()
    if DEBUG_DUMP:
        d_qt = nc.dram_tensor("d_qt", [128, L], F16, kind="ExternalOutput").ap()
        d_kt = nc.dram_tensor("d_kt", [128, L // 2], F16, kind="ExternalOutput").ap()
        d_v = nc.dram_tensor("d_v", [128, (L // 128) * 66], F16, kind="ExternalOutput").ap()
        d_pst = nc.dram_tensor("d_pst", [128, 1024], F32, kind="ExternalOutput").ap()
        d_y1 = nc.dram_tensor("d_y1", [128, 1024], I16, kind="ExternalOutput").ap()
        d_pp = nc.dram_tensor("d_pp", [128, 1024], F16, kind="ExternalOutput").ap()
        d_poq = nc.dram_tensor("d_poq", [128, 264], F32, kind="ExternalOutput").ap()


    ADD = mybir.AluOpType.add
    MULT = mybir.AluOpType.mult

    with tile.TileContext(nc) as tc:
        with (
            tc.tile_pool(name="const", bufs=1) as constp,
            tc.tile_pool(name="persist", bufs=1) as persist,
            tc.tile_pool(name="kin", bufs=3) as kinp,
            tc.tile_pool(name="vin", bufs=2) as vinp,
            tc.tile_pool(name="qin", bufs=3) as qinp,
            tc.tile_pool(name="mask", bufs=2) as mpool,
            tc.tile_pool(name="y1p", bufs=3) as y1pool,
            tc.tile_pool(name="y2p", bufs=3) as y2pool,
            tc.tile_pool(name="pp", bufs=6) as ppool,
            tc.tile_pool(name="osb", bufs=2) as ospool,
            tc.tile_pool(name="zinv", bufs=4) as zpool,
            tc.tile_pool(name="otile", bufs=4) as otpool,
            tc.tile_pool(name="ps_st", bufs=2, space="PSUM") as ps_st,
            tc.tile_pool(name="ps_o", bufs=2, space="PSUM") as ps_o,
            tc.tile_pool(name="ps_small", bufs=2, space="PSUM") as ps_small,
        ):
            wq_sb = constp.tile([128, 8, 128], F16)
            wk_sb = constp.tile([128, 8, 128], F16)
            wv_sb = constp.tile([128, 8, H], F16)
            nc.sync.dma_start(wq_sb[:], wqP.rearrange("p (c h) -> p c h", h=128))
            nc.sync.dma_start(wk_sb[:], wkP.rearrange("p (c h) -> p c h", h=128))
            nc.sync.dma_start(wv_sb[:], wvP.rearrange("p (c h) -> p c h", h=H))

            QT_sb = persist.tile([128, L], F16)
            KT_sb = persist.tile([128, L // 2], F16)
            V_sb = persist.tile([128, L // 128, 66], F16)
            nc.vector.memset(V_sb[:, :, 64:66], 0.0)
            nc.vector.memset(V_sb[:, :, 64:65], 1.0)

            # -------- Phase-1 helpers (K/V projections, folded into qb0) --------
            def load_k(lb):
                ls = lb * LB
                k_in = kinp.tile([128, 8, LB], F8, tag="kin", name="k_in")
                nc.sync.dma_start(k_in[:], kP[:, lb, :, :])
                return k_in

            def load_v(lb):
                ls = lb * LB
                v_in = vinp.tile([128, 8, LB], F16, tag="vin", name="v_in")
                nc.sync.dma_start(v_in[:], vP[:, lb, :, :])
                return v_in

            def proj_k(lb, k_in):
                p_k = ps_small.tile([128, LB], F32, tag="small", name="p_k")
                for ec in range(8):
                    nc.tensor.matmul(
                        p_k[:], wk_sb[:, ec, :], k_in[:, ec, :],
                        start=(ec == 0), stop=(ec == 7),
                    )
                pk_e = p_k[0:64, :].rearrange("p (c x) -> p c x", x=128)
                pk_o = p_k[64:128, :].rearrange("p (c x) -> p c x", x=128)
                kt_e = KT_sb[0:64, 256 * lb : 256 * lb + 256].rearrange(
                    "p (c x) -> p c x", x=128
                )
                kt_o = KT_sb[64:128, 256 * lb : 256 * lb + 256].rearrange(
                    "p (c x) -> p c x", x=128
                )
                nc.vector.tensor_scalar_mul(kt_e, pk_e[:, 0::2, :], 1.0)
                nc.vector.tensor_scalar_mul(kt_o, pk_o[:, 1::2, :], 1.0)

            def proj_v_chunk(c, v_in):
                sub = c % 4
                p_v = ps_small.tile([128, H], F32, tag="small", name="p_v")
                for ec in range(8):
                    nc.tensor.matmul(
                        p_v[:],
                        v_in[:, ec, sub * 128 : (sub + 1) * 128],
                        wv_sb[:, ec, :],
                        start=(ec == 0),
                        stop=(ec == 7),
                    )
                nc.vector.tensor_scalar_mul(V_sb[:, c, 0:H], p_v[:], 1.0)

            # ---------------- Phase 2 helpers ----------------
            def load_q(qb):
                qs = qb * LB
                q_in = qinp.tile([128, 8, LB], F8, tag="qin")
                nc.sync.dma_start(q_in[:], qP[:, qb, :, :])
                return q_in

            def proj_q_half(qb, q_in, half, p_q):
                qs = qb * LB
                for ec in range(4 * half, 4 * half + 4):
                    nc.tensor.matmul(
                        p_q[:], wq_sb[:, ec, :], q_in[:, ec, :],
                        start=(ec == 0), stop=(ec == 7),
                    )
                if half == 1:
                    nc.vector.tensor_scalar_mul(
                        QT_sb[:, qs : qs + LB], p_q[:], 1.0
                    )

            def proj_q(qb, q_in):
                p_q = ps_small.tile([128, LB], F32, tag="small", name="p_q")
                proj_q_half(qb, q_in, 0, p_q)
                proj_q_half(qb, q_in, 1, p_q)

            def alloc_mask():
                return mpool.tile([128, L // 128, LB], F16, tag="m", name="mtile")

            def load_mask_slice(m, qb, c0, c1):
                nc.gpsimd.dma_start(
                    m[:, c0:c1, :], mP[:, qb, c0:c1, :]
                )

            def load_mask_quarter(m, qb, qtr):
                load_mask_slice(m, qb, 8 * qtr, 8 * (qtr + 1))

            def epilogue_start(qb, p_o):
                o_sb = ospool.tile([66, LB], F32, tag="osb")
                nc.vector.tensor_scalar_mul(o_sb[:], p_o[0:66, :], 1.0)
                return o_sb

            def epilogue_sub(qb, o_sb, sub):
                p_t = ps_small.tile([128, 66], F32, tag="small")
                nc.tensor.transpose(
                    p_t[:], o_sb[:, sub * 128 : (sub + 1) * 128],
                    id32_sb[0:66, 0:66],
                )
                zinv = zpool.tile([128, 1], F32, tag="z")
                nc.vector.reciprocal(zinv[:], p_t[:, 64:65])
                ot = otpool.tile([128, H], F32, tag="ot")
                nc.vector.tensor_scalar_mul(ot[:], p_t[:, 0:H], zinv[:])
                r0 = qb * LB + sub * 128
                nc.sync.dma_start(out[r0 : r0 + 128, :], ot[:])

            def epilogue(qb, p_o):
                o_sb = epilogue_start(qb, p_o)
                for sub in range(4):
                    epilogue_sub(qb, o_sb, sub)

            k0 = load_k(0)
            q0 = load_q(0)
            k_nxt = load_k(1)
            proj_k(0, k0)
            k_cur = k_nxt
            proj_q(0, q0)
            m0 = alloc_mask()
            load_mask_slice(m0, 0, 0, 4)
            load_mask_slice(m0, 0, 4, 8)
            for qtr in range(1, 4):
                load_mask_quarter(m0, 0, qtr)
            v_cur = load_v(0)
            v_nxt = None
            q_next = load_q(1)


            pending = None
            mtile = m0
            for qb in range(NQB):
                qs = qb * LB
                p_o = ps_o.tile([128, LB], F32, tag="p_o")
                m_flat = mtile[:].rearrange("p c q -> p (c q)")
                av_q = []
                m_next = None
                for j in range(16):
                    p_st = ps_st.tile([128, 1024], F32, tag="p_st")
                    nc.tensor.matmul(
                        p_st[:, 0:LB],
                        KT_sb[0:64, j * 128 : (j + 1) * 128],
                        QT_sb[0:64, qs : qs + LB],
                        start=True, stop=True, tile_position=(0, 0),
                    )
                    nc.tensor.matmul(
                        p_st[:, LB : 2 * LB],
                        KT_sb[64:128, j * 128 : (j + 1) * 128],
                        QT_sb[64:128, qs : qs + LB],
                        start=True, stop=True, tile_position=(64, 0),
                    )
                    if qb == 0:
                        if j % 2 == 0 and j // 2 + 1 < NQB:
                            proj_k(j // 2 + 1, k_cur)
                            if j // 2 + 2 < NQB:
                                k_cur = load_k(j // 2 + 2)
                        for c in (2 * j, 2 * j + 1):
                            if c % 4 == 0:
                                if c // 4 + 1 < NQB:
                                    v_nxt = load_v(c // 4 + 1)
                            proj_v_chunk(c, v_cur)
                            if c % 4 == 3:
                                v_cur = v_nxt
                    if j == 1 and pending is not None:
                        epilogue(*pending)
                        pending = None
                    if qb + 1 < NQB and j in (1, 3, 5, 7):
                        if m_next is None:
                            m_next = alloc_mask()
                        load_mask_quarter(m_next, qb + 1, (j - 1) // 2)
                    if j == 8 and qb + 1 < NQB:
                        if qb == 0:
                            proj_q(qb + 1, q_next)
                        else:
                            p_qn = ps_small.tile(
                                [128, LB], F32, tag="small", name="p_qn"
                            )
                            proj_q_half(qb + 1, q_next, 0, p_qn)
                    if j == 10 and qb + 1 < NQB and qb > 0:
                        proj_q_half(qb + 1, q_next, 1, p_qn)
                    if j == 12 and qb + 2 < NQB:
                        q_next = load_q(qb + 2)

                    if len(av_q) >= 2:
                        ppv, jv = av_q.pop(0)
                        nc.tensor.matmul(
                            p_o[0:66, :], V_sb[:, 2 * jv, :], ppv[:, 0:LB],
                            start=(jv == 0), stop=False, skip_group_check=True,
                        )
                        nc.tensor.matmul(
                            p_o[0:66, :], V_sb[:, 2 * jv + 1, :], ppv[:, LB : 2 * LB],
                            start=False, stop=False, skip_group_check=True,
                        )
                    pe = ppool.tile([128, 2 * LB], F16, tag="pe")
                    nc.scalar.activation(
                        pe[:], p_st[:], mybir.ActivationFunctionType.Exp, scale=0.125
                    )
                    pp = ppool.tile([128, 2 * LB], F16, tag="pp")
                    nc.vector.tensor_mul(
                        pp[:], pe[:], m_flat[:, j * 1024 : (j + 1) * 1024]
                    )
                    av_q.append((pp, j))
                for idx, (ppv, jv) in enumerate(av_q):
                    nc.tensor.matmul(
                        p_o[0:66, :], V_sb[:, 2 * jv, :], ppv[:, 0:LB],
                        start=False, stop=False, skip_group_check=True,
                    )
                    nc.tensor.matmul(
                        p_o[0:66, :], V_sb[:, 2 * jv + 1, :], ppv[:, LB : 2 * LB],
                        start=False, stop=(idx == len(av_q) - 1),
                        skip_group_check=True,
                    )
                pending = (qb, p_o)
                mtile = m_next
            epilogue(*pending)
            if DEBUG_DUMP:
                nc.sync.dma_start(d_qt, QT_sb[:])
                nc.sync.dma_start(d_kt, KT_sb[:])
                nc.sync.dma_start(d_v, V_sb[:].rearrange("p a b -> p (a b)"))
    nc.compile()
    return nc


_NC_CACHE = {}


def kernel(query, key, value, mask, WQ, WK, WV):
    if "nc" not in _NC_CACHE:
        _NC_CACHE["nc"] = build_nc()
    nc = _NC_CACHE["nc"]

    f8 = ml_dtypes.float8_e4m3

    def pack_w(w2):
        # [E, M] -> [128, 8*M] partition-major (p, c, h)
        m = w2.shape[1]
        return np.ascontiguousarray(
            w2.reshape(8, 128, m).transpose(1, 0, 2).reshape(128, 8 * m)
        )

    def pack_in(xT, blocks):
        # [E or L, L] -> [128, blocks_outer, inner_c, LB] partition-major
        n0 = xT.shape[0] // 128
        return np.ascontiguousarray(
            xT.reshape(n0, 128, blocks, L // blocks).transpose(1, 2, 0, 3)
        )

    wq2 = np.asarray(WQ, np.float32).T
    wq2 = np.concatenate([wq2, wq2], axis=1).astype(np.float16)
    wk2 = np.asarray(WK, np.float32).T
    wk2 = np.concatenate([wk2, wk2], axis=1).astype(np.float16)
    wvt = np.asarray(WV, np.float16).T
    wq2, wk2, wvt = pack_w(wq2), pack_w(wk2), pack_w(wvt)
    notm = (~np.asarray(mask)).astype(np.uint8)
    id32 = np.eye(128, dtype=np.float32)
    in_maps = []
    for b in range(B):
        in_maps.append(
            {
                "qP": pack_in(np.asarray(query[b]).T.astype(f8), 8),
                "kP": pack_in(np.asarray(key[b]).T.astype(f8), 8),
                "vP": pack_in(np.asarray(value[b], np.float16).T, 8),
                "mP": pack_in(notm[b].T, 8),
                "ident32": id32,
                "wqP": wq2,
                "wkP": wk2,
                "wvP": wvt,
            }
        )
    res = bass_utils.run_bass_kernel_spmd(nc, in_maps, core_ids=list(range(NCORES)))
    return np.stack([res.results[b]["out"] for b in range(B)], axis=0)


if __name__ == "__main__":
    rng = np.random.default_rng(0)
    q = rng.standard_normal((B, L, E), dtype=np.float32)
    k = rng.standard_normal((B, L, E), dtype=np.float32)
    v = rng.standard_normal((B, L, E), dtype=np.float32)
    m = rng.integers(0, 2, size=(B, L, L)).astype(bool)
    s = 1.0 / np.sqrt(E)
    wq = rng.uniform(-s, s, size=(H, E)).astype(np.float32)
    wk = rng.uniform(-s, s, size=(H, E)).astype(np.float32)
    wv = rng.uniform(-s, s, size=(H, E)).astype(np.float32)
    o = kernel(query=q, key=k, value=v, mask=m, WQ=wq, WK=wk, WV=wv)
    print(o.shape, o.dtype)


# revision 24
# speedup vs baseline: 1.8666x; 1.0285x over previous
"""Fused masked-attention for Trainium2, data-parallel over batch on 8 cores.

Per core (one batch element, L=4096, E=1024, H=64):
  Q/K projections from fp8e4m3 inputs (host cast, free); V from fp16 input.
  Weights fp16, host-duplicated [W^T|W^T] so the score matmuls' row-tiled
  halves (tile_position (0,0)/(64,0), K=64 each, even/odd chunk layout in
  KT_sb) read their stationary/moving operands at the right partitions.
  Scores accumulate in PSUM f32; ACT computes exp(0.125*s) -> fp16; the
  mask arrives as u8 {0,1}, SWDGE-cast to fp16 during DMA, and is applied
  by one DVE f16x f16 multiply (2x packed mode). AV uses V_ext=[V|1] fp16
  as stationary (out [h+Z, q]); AV matmuls are software-pipelined 2 pairs
  behind exp so the in-order PE queue never stalls on ACT. Epilogue:
  PE transpose to [q, h+Z], DVE reciprocal of Z, scale, DMA out.

HBM per core: q/k fp8 (4MB each), v fp16 (8MB), mask u8 (16MB), out 1MB.
"""

import numpy as np
import ml_dtypes

import concourse.bass as bass
import concourse.tile as tile
from concourse import bacc, mybir
from concourse import bass_utils

B, L, E, H = 8, 4096, 1024, 64
NCORES = 8
F32 = mybir.dt.float32
F16 = mybir.dt.float16
F8 = mybir.dt.float8e4
I16 = mybir.dt.int16
U8 = mybir.dt.uint8

LB = 512
NQB = L // LB
B1 = 15104.0
IDENT = mybir.ActivationFunctionType.Identity
DEBUG_DUMP = False


def build_nc():
    nc = bacc.Bacc(
        "TRN2",
        target_bir_lowering=False,
        debug=False,
        enable_asserts=False,
        num_devices=NCORES,
    )
    qP = nc.dram_tensor("qP", [128, 8, 8, LB], F8, kind="ExternalInput").ap()
    kP = nc.dram_tensor("kP", [128, 8, 8, LB], F8, kind="ExternalInput").ap()
    vP = nc.dram_tensor("vP", [128, 8, 8, LB], F16, kind="ExternalInput").ap()
    mP = nc.dram_tensor("mP", [128, 8, 32, LB], U8, kind="ExternalInput").ap()
    wqP = nc.dram_tensor("wqP", [128, 8 * 128], F16, kind="ExternalInput").ap()
    wkP = nc.dram_tensor("wkP", [128, 8 * 128], F16, kind="ExternalInput").ap()
    wvP = nc.dram_tensor("wvP", [128, 8 * H], F16, kind="ExternalInput").ap()
    out = nc.dram_tensor("out", [L, H], F32, kind="ExternalOutput").ap()
    if DEBUG_DUMP:
        d_qt = nc.dram_tensor("d_qt", [128, L], F16, kind="ExternalOutput").ap()
        d_kt = nc.dram_tensor("d_kt", [128, L // 2], F16, kind="ExternalOutput").ap()
        d_v = nc.dram_tensor("d_v", [128, (L // 128) * 66], F16, kind="ExternalOutput").ap()
        d_pst = nc.dram_tensor("d_pst", [128, 1024], F32, kind="ExternalOutput").ap()
        d_y1 = nc.dram_tensor("d_y1", [128, 1024], I16, kind="ExternalOutput").ap()
        d_pp = nc.dram_tensor("d_pp", [128, 1024], F16, kind="ExternalOutput").ap()
        d_poq = nc.dram_tensor("d_poq", [128, 264], F32, kind="ExternalOutput").ap()


    ADD = mybir.AluOpType.add
    MULT = mybir.AluOpType.mult

    with tile.TileContext(nc) as tc:
        with (
            tc.tile_pool(name="const", bufs=1) as constp,
            tc.tile_pool(name="persist", bufs=1) as persist,
            tc.tile_pool(name="kin", bufs=3) as kinp,
            tc.tile_pool(name="vin", bufs=2) as vinp,
            tc.tile_pool(name="qin", bufs=3) as qinp,
            tc.tile_pool(name="mask", bufs=2) as mpool,
            tc.tile_pool(name="y1p", bufs=3) as y1pool,
            tc.tile_pool(name="y2p", bufs=3) as y2pool,
            tc.tile_pool(name="pp", bufs=6) as ppool,
            tc.tile_pool(name="osb", bufs=2) as ospool,
            tc.tile_pool(name="zinv", bufs=4) as zpool,
            tc.tile_pool(name="otile", bufs=4) as otpool,
            tc.tile_pool(name="ps_st", bufs=2, space="PSUM") as ps_st,
            tc.tile_pool(name="ps_o", bufs=2, space="PSUM") as ps_o,
            tc.tile_pool(name="ps_small", bufs=2, space="PSUM") as ps_small,
        ):
            wq_sb = constp.tile([128, 8, 128], F16)
            wk_sb = constp.tile([128, 8, 128], F16)
            wv_sb = constp.tile([128, 8, H], F16)
            nc.sync.dma_start(wq_sb[:], wqP.rearrange("p (c h) -> p c h", h=128))
            nc.sync.dma_start(wk_sb[:], wkP.rearrange("p (c h) -> p c h", h=128))
            nc.sync.dma_start(wv_sb[:], wvP.rearrange("p (c h) -> p c h", h=H))

            QT_sb = persist.tile([128, L], F16)
            KT_sb = persist.tile([128, L // 2], F16)
            V_sb = persist.tile([128, L // 128, 66], F16)
            nc.vector.memset(V_sb[:, :, 64:66], 0.0)
            nc.vector.memset(V_sb[:, :, 64:65], 1.0)

            # -------- Phase-1 helpers (K/V projections, folded into qb0) --------
            def load_k(lb):
                ls = lb * LB
                k_in = kinp.tile([128, 8, LB], F8, tag="kin", name="k_in")
                nc.sync.dma_start(k_in[:], kP[:, lb, :, :])
                return k_in

            def load_v(lb):
                ls = lb * LB
                v_in = vinp.tile([128, 8, LB], F16, tag="vin", name="v_in")
                nc.sync.dma_start(v_in[:], vP[:, lb, :, :])
                return v_in

            def proj_k(lb, k_in):
                p_k = ps_small.tile([128, LB], F32, tag="small", name="p_k")
                for ec in range(8):
                    nc.tensor.matmul(
                        p_k[:], wk_sb[:, ec, :], k_in[:, ec, :],
                        start=(ec == 0), stop=(ec == 7),
                    )
                pk_e = p_k[0:64, :].rearrange("p (c x) -> p c x", x=128)
                pk_o = p_k[64:128, :].rearrange("p (c x) -> p c x", x=128)
                kt_e = KT_sb[0:64, 256 * lb : 256 * lb + 256].rearrange(
                    "p (c x) -> p c x", x=128
                )
                kt_o = KT_sb[64:128, 256 * lb : 256 * lb + 256].rearrange(
                    "p (c x) -> p c x", x=128
                )
                nc.vector.tensor_scalar_mul(kt_e, pk_e[:, 0::2, :], 1.0)
                nc.vector.tensor_scalar_mul(kt_o, pk_o[:, 1::2, :], 1.0)

            def proj_v_chunk(c, v_in):
                sub = c % 4
                p_v = ps_small.tile([128, H], F32, tag="small", name="p_v")
                for ec in range(8):
                    nc.tensor.matmul(
                        p_v[:],
                        v_in[:, ec, sub * 128 : (sub + 1) * 128],
                        wv_sb[:, ec, :],
                        start=(ec == 0),
                        stop=(ec == 7),
                    )
                nc.vector.tensor_scalar_mul(V_sb[:, c, 0:H], p_v[:], 1.0)

            # ---------------- Phase 2 helpers ----------------
            def load_q(qb):
                qs = qb * LB
                q_in = qinp.tile([128, 8, LB], F8, tag="qin")
                nc.sync.dma_start(q_in[:], qP[:, qb, :, :])
                return q_in

            def proj_q_half(qb, q_in, half, p_q):
                qs = qb * LB
                for ec in range(4 * half, 4 * half + 4):
                    nc.tensor.matmul(
                        p_q[:], wq_sb[:, ec, :], q_in[:, ec, :],
                        start=(ec == 0), stop=(ec == 7),
                    )
                if half == 1:
                    nc.vector.tensor_scalar_mul(
                        QT_sb[:, qs : qs + LB], p_q[:], 1.0
                    )

            def proj_q(qb, q_in):
                p_q = ps_small.tile([128, LB], F32, tag="small", name="p_q")
                proj_q_half(qb, q_in, 0, p_q)
                proj_q_half(qb, q_in, 1, p_q)

            def alloc_mask():
                return mpool.tile([128, L // 128, LB], F16, tag="m", name="mtile")

            def load_mask_quarter(m, qb, qtr):
                qs = qb * LB
                nc.gpsimd.dma_start(
                    m[:, 8 * qtr : 8 * (qtr + 1), :],
                    mP[:, qb, 8 * qtr : 8 * (qtr + 1), :],
                )

            def epilogue_start(qb, p_o):
                o_sb = ospool.tile([66, LB], F32, tag="osb")
                nc.vector.tensor_scalar_mul(o_sb[:], p_o[0:66, :], 1.0)
                return o_sb

            def epilogue_sub(qb, o_sb, sub):
                p_t = ps_small.tile([128, 66], F32, tag="small")
                nc.tensor.transpose(
                    p_t[:], o_sb[:, sub * 128 : (sub + 1) * 128],
                    id32_sb[0:66, 0:66],
                )
                zinv = zpool.tile([128, 1], F32, tag="z")
                nc.vector.reciprocal(zinv[:], p_t[:, 64:65])
                ot = otpool.tile([128, H], F32, tag="ot")
                nc.vector.tensor_scalar_mul(ot[:], p_t[:, 0:H], zinv[:])
                r0 = qb * LB + sub * 128
                nc.sync.dma_start(out[r0 : r0 + 128, :], ot[:])

            def epilogue(qb, p_o):
                o_sb = epilogue_start(qb, p_o)
                for sub in range(4):
                    epilogue_sub(qb, o_sb, sub)

            k0 = load_k(0)
            q0 = load_q(0)
            k_nxt = load_k(1)
            proj_k(0, k0)
            k_cur = k_nxt
            proj_q(0, q0)
            m0 = alloc_mask()
            for qtr in range(4):
                load_mask_quarter(m0, 0, qtr)
            v_cur = load_v(0)
            v_nxt = None
            q_next = load_q(1)


            pending = None
            mtile = m0
            for qb in range(NQB):
                qs = qb * LB
                p_o = ps_o.tile([128, LB], F32, tag="p_o")
                m_flat = mtile[:].rearrange("p c q -> p (c q)")
                av_q = []
                m_next = None
                for j in range(16):
                    p_st = ps_st.tile([128, 1024], F32, tag="p_st")
                    nc.tensor.matmul(
                        p_st[:, 0:LB],
                        KT_sb[0:64, j * 128 : (j + 1) * 128],
                        QT_sb[0:64, qs : qs + LB],
                        start=True, stop=True, tile_position=(0, 0),
                    )
                    nc.tensor.matmul(
                        p_st[:, LB : 2 * LB],
                        KT_sb[64:128, j * 128 : (j + 1) * 128],
                        QT_sb[64:128, qs : qs + LB],
                        start=True, stop=True, tile_position=(64, 0),
                    )
                    if qb == 0:
                        if j % 2 == 0 and j // 2 + 1 < NQB:
                            proj_k(j // 2 + 1, k_cur)
                            if j // 2 + 2 < NQB:
                                k_cur = load_k(j // 2 + 2)
                        for c in (2 * j, 2 * j + 1):
                            if c % 4 == 0:
                                if c // 4 + 1 < NQB:
                                    v_nxt = load_v(c // 4 + 1)
                            proj_v_chunk(c, v_cur)
                            if c % 4 == 3:
                                v_cur = v_nxt
                    if j == 1 and pending is not None:
                        epilogue(*pending)
                        pending = None
                    if qb + 1 < NQB and j in (1, 3, 5, 7):
                        if m_next is None:
                            m_next = alloc_mask()
                        load_mask_quarter(m_next, qb + 1, (j - 1) // 2)
                    if j == 8 and qb + 1 < NQB:
                        if qb == 0:
                            proj_q(qb + 1, q_next)
                        else:
                            p_qn = ps_small.tile(
                                [128, LB], F32, tag="small", name="p_qn"
                            )
                            proj_q_half(qb + 1, q_next, 0, p_qn)
                    if j == 10 and qb + 1 < NQB and qb > 0:
                        proj_q_half(qb + 1, q_next, 1, p_qn)
                    if j == 12 and qb + 2 < NQB:
                        q_next = load_q(qb + 2)

                    if len(av_q) >= 2:
                        ppv, jv = av_q.pop(0)
                        nc.tensor.matmul(
                            p_o[0:66, :], V_sb[:, 2 * jv, :], ppv[:, 0:LB],
                            start=(jv == 0), stop=False, skip_group_check=True,
                        )
                        nc.tensor.matmul(
                            p_o[0:66, :], V_sb[:, 2 * jv + 1, :], ppv[:, LB : 2 * LB],
                            start=False, stop=False, skip_group_check=True,
                        )
                    pe = ppool.tile([128, 2 * LB], F16, tag="pe")
                    nc.scalar.activation(
                        pe[:], p_st[:], mybir.ActivationFunctionType.Exp, scale=0.125
                    )
                    pp = ppool.tile([128, 2 * LB], F16, tag="pp")
                    nc.vector.tensor_mul(
                        pp[:], pe[:], m_flat[:, j * 1024 : (j + 1) * 1024]
                    )
                    av_q.append((pp, j))
                for idx, (ppv, jv) in enumerate(av_q):
                    nc.tensor.matmul(
                        p_o[0:66, :], V_sb[:, 2 * jv, :], ppv[:, 0:LB],
                        start=False, stop=False, skip_group_check=True,
                    )
                    nc.tensor.matmul(
                        p_o[0:66, :], V_sb[:, 2 * jv + 1, :], ppv[:, LB : 2 * LB],
                        start=False, stop=(idx == len(av_q) - 1),
                        skip_group_check=True,
                    )
                pending = (qb, p_o)
                mtile = m_next
            epilogue(*pending)
            if DEBUG_DUMP:
                nc.sync.dma_start(d_qt, QT_sb[:])
                nc.sync.dma_start(d_kt, KT_sb[:])
                nc.sync.dma_start(d_v, V_sb[:].rearrange("p a b -> p (a b)"))
    nc.compile()
    return nc


_NC_CACHE = {}


def kernel(query, key, value, mask, WQ, WK, WV):
    if "nc" not in _NC_CACHE:
        _NC_CACHE["nc"] = build_nc()
    nc = _NC_CACHE["nc"]

    f8 = ml_dtypes.float8_e4m3

    def pack_w(w2):
        # [E, M] -> [128, 8*M] partition-major (p, c, h)
        m = w2.shape[1]
        return np.ascontiguousarray(
            w2.reshape(8, 128, m).transpose(1, 0, 2).reshape(128, 8 * m)
        )

    def pack_in(xT, blocks):
        # [E or L, L] -> [128, blocks_outer, inner_c, LB] partition-major
        n0 = xT.shape[0] // 128
        return np.ascontiguousarray(
            xT.reshape(n0, 128, blocks, L // blocks).transpose(1, 2, 0, 3)
        )

    wq2 = np.asarray(WQ, np.float32).T
    wq2 = np.concatenate([wq2, wq2], axis=1).astype(np.float16)
    wk2 = np.asarray(WK, np.float32).T
    wk2 = np.concatenate([wk2, wk2], axis=1).astype(np.float16)
    wvt = np.asarray(WV, np.float16).T
    wq2, wk2, wvt = pack_w(wq2), pack_w(wk2), pack_w(wvt)
    notm = (~np.asarray(mask)).astype(np.uint8)
    id32 = np.eye(128, dtype=np.float32)
    in_maps = []
    for b in range(B):
        in_maps.append(
            {
                "qP": pack_in(np.asarray(query[b]).T.astype(f8), 8),
                "kP": pack_in(np.asarray(key[b]).T.astype(f8), 8),
                "vP": pack_in(np.asarray(value[b], np.float16).T, 8),
                "mP": pack_in(notm[b].T, 8),
                "ident32": id32,
                "wqP": wq2,
                "wkP": wk2,
                "wvP": wvt,
            }
        )
    res = bass_utils.run_bass_kernel_spmd(nc, in_maps, core_ids=list(range(NCORES)))
    return np.stack([res.results[b]["out"] for b in range(B)], axis=0)


if __name__ == "__main__":
    rng = np.random.default_rng(0)
    q = rng.standard_normal((B, L, E), dtype=np.float32)
    k = rng.standard_normal((B, L, E), dtype=np.float32)
    v = rng.standard_normal((B, L, E), dtype=np.float32)
    m = rng.integers(0, 2, size=(B, L, L)).astype(bool)
    s = 1.0 / np.sqrt(E)
    wq = rng.uniform(-s, s, size=(H, E)).astype(np.float32)
    wk = rng.uniform(-s, s, size=(H, E)).astype(np.float32)
    wv = rng.uniform(-s, s, size=(H, E)).astype(np.float32)
    o = kernel(query=q, key=k, value=v, mask=m, WQ=wq, WK=wk, WV=wv)
    print(o.shape, o.dtype)
